# revision 3
# baseline (speedup 1.0000x reference)
"""MoE (8 experts, top-2, D=1024, F=4096, T=8192) on 8 NeuronCores.

Strategy: expert-parallel — core c holds expert c's FFN weights (bf16), every
core runs the dense FFN over all tokens, computes the fp32 gating/top-2 mask
on-device, scales its expert's output by the masked gate prob, and writes a
weighted partial [T, D]. The host sums the 8 partials (the unshard step).
"""

import os
import sys

for _p in ("/opt/trn_rl_repo", os.path.expanduser("~/.axon_site/_ro/trn_rl_repo")):
    if os.path.isdir(_p) and _p not in sys.path:
        sys.path.insert(0, _p)

import numpy as np
import ml_dtypes

import concourse.bass as bass
import concourse.tile as tile
from concourse import mybir
from concourse.bass_utils import run_bass_kernel_spmd
from concourse.vector_clock import ScopedClock

# ---------------------------------------------------------------------------
# Workaround: the pinned walrus rejects >1 sync-wait per instruction, but
# Tile's stock tail-drain aggregates one wait per logical proc onto a single
# Drain. Split the waits across chained drains (same semantics: conjunction).
def _split_drain_and_barrier(self, tick_clock, wait_clock):
    drain_inst = self.nc.sync.drain()
    wait_clock.add_sem_waits(
        drain_inst.ins, ScopedClock({None: tick_clock.global_clock})
    )
    si = drain_inst.ins.sync_info
    if si is not None and si.on_wait and len(si.on_wait) > 1:
        waits = list(si.on_wait)
        drain_inst.ins.sync_info = mybir.SyncInfo(
            on_wait=[waits[0]], on_update=list(si.on_update)
        )
        for w in waits[1:]:
            extra = self.nc.sync.drain()
            extra.ins.sync_info = mybir.SyncInfo(on_wait=[w], on_update=[])
    self.nc.all_engine_barrier()
    popped = self.nc._tile_sem_poison_stack.pop()
    assert popped is self._sem_poison
    self.nc.clear_and_free_semaphores(list(self.sems.allocated().values()))
    self.nc.all_engine_barrier()


tile.TileContext._drain_and_barrier = _split_drain_and_barrier

# Same walrus limitation, general case: any instruction whose sem-assignment
# produced >1 on_wait gets the extra waits hoisted onto NoOps emitted just
# before it on the same engine (same-engine program order makes this
# equivalent — the conjunction of waits is satisfied before the instruction).
import json as _json

_orig_to_json_bytes = bass.Bass.to_json_bytes


def _to_json_bytes_split_waits(self):
    raw = _orig_to_json_bytes(self)
    d = _json.loads(raw)
    changed = False
    for fn in d.get("functions", []):
        for b in fn.get("blocks", []):
            out = []
            for i in b.get("instructions", []):
                si = i.get("sync_info")
                waits = (si or {}).get("on_wait") or []
                if len(waits) > 1:
                    changed = True
                    for k, w in enumerate(waits[:-1]):
                        out.append(
                            {
                                "name": f"{i['name']}-wsplit{k}",
                                "opcode": "NoOp",
                                "engine": i.get("engine"),
                                "ins": [],
                                "outs": [],
                                "sync_info": {"on_wait": [w], "on_update": []},
                            }
                        )
                    si["on_wait"] = [waits[-1]]
                out.append(i)
            b["instructions"] = out
    if not changed:
        return raw
    return _json.dumps(d).encode()


bass.Bass.to_json_bytes = _to_json_bytes_split_waits
# ---------------------------------------------------------------------------

E = 8
TOPK = 2
D = 1024
F = 4096
T = 8192  # BATCH * SEQ
P = 128
TB = 256  # tokens per block
NTB = T // TB
TSUB = TB // P  # 128-token sub-tiles per block
DS = D // P  # d-strips
FS = F // P  # f-strips
NCORES = 8

F32 = mybir.dt.float32
BF16 = mybir.dt.bfloat16
AX = mybir.AxisListType.X
ALU = mybir.AluOpType
ACTF = mybir.ActivationFunctionType

_CACHE = {}
LAST = {}


def _build_nc():
    nc = bass.Bass()
    xT = nc.dram_tensor("xT", [D, T], F32, kind="ExternalInput")
    w1 = nc.dram_tensor("w1", [D, F], BF16, kind="ExternalInput")
    w2 = nc.dram_tensor("w2", [F, D], BF16, kind="ExternalInput")
    b1 = nc.dram_tensor("b1", [F], F32, kind="ExternalInput")
    b2r = nc.dram_tensor("b2r", [P, D], F32, kind="ExternalInput")
    wg = nc.dram_tensor("wg", [D, E], F32, kind="ExternalInput")
    bgr = nc.dram_tensor("bgr", [P, E], F32, kind="ExternalInput")
    sel = nc.dram_tensor("sel", [P, E], F32, kind="ExternalInput")
    out = nc.dram_tensor("out", [T, D], F32, kind="ExternalOutput")

    xT_t = xT.rearrange("(o p) t -> p o t", p=P)
    w1_t = w1.rearrange("(o p) f -> p o f", p=P)
    w2_t = w2.rearrange("(o p) d -> p o d", p=P)
    b1_t = b1.rearrange("(o p) -> p o", p=P)
    wg_t = wg.rearrange("(o p) e -> p o e", p=P)

    with tile.TileContext(nc) as tc:
        with (
            tc.tile_pool(name="const", bufs=1) as cpool,
            tc.tile_pool(name="xt", bufs=2) as xt_pool,
            tc.tile_pool(name="xtbf", bufs=2) as xtbf_pool,
            tc.tile_pool(name="h", bufs=2) as h_pool,
            tc.tile_pool(name="gate", bufs=2) as g_pool,
            tc.tile_pool(name="osb", bufs=4) as o_pool,
            tc.tile_pool(name="psh", bufs=2, space="PSUM") as psh_pool,
            tc.tile_pool(name="pso", bufs=2, space="PSUM") as pso_pool,
            tc.tile_pool(name="psg", bufs=2, space="PSUM") as psg_pool,
        ):
            w1_sb = cpool.tile([P, DS, F], BF16)
            for ds in range(DS):
                nc.sync.dma_start(w1_sb[:, ds, :], w1_t[:, ds, :])
            w2_sb = cpool.tile([P, FS, D], BF16)
            for fs in range(FS):
                nc.sync.dma_start(w2_sb[:, fs, :], w2_t[:, fs, :])
            b1_sb = cpool.tile([P, FS], F32)
            nc.sync.dma_start(b1_sb[:], b1_t)
            b2_sb = cpool.tile([P, D], F32)
            nc.sync.dma_start(b2_sb[:], b2r[:])
            wg_sb = cpool.tile([P, DS, E], F32)
            nc.sync.dma_start(wg_sb[:], wg_t)
            bg_sb = cpool.tile([P, E], F32)
            nc.sync.dma_start(bg_sb[:], bgr[:])
            sel_sb = cpool.tile([P, E], F32)
            nc.sync.dma_start(sel_sb[:], sel[:])

            for tb in range(NTB):
                t0 = tb * TB
                xt = xt_pool.tile([P, DS, TB], F32)
                nc.sync.dma_start(xt[:], xT_t[:, :, t0 : t0 + TB])
                xtbf = xtbf_pool.tile([P, DS, TB], BF16)
                nc.vector.tensor_copy(xtbf[:], xt[:])

                # --- gating: fp32 logits -> softmax probs, top-2 mask,
                # select this core's expert column -> wcol [P, TSUB]
                wcol = g_pool.tile([P, TSUB], F32, name="wcol")
                for ts in range(TSUB):
                    gps = psg_pool.tile([P, E], F32)
                    for ds in range(DS):
                        nc.tensor.matmul(
                            gps[:],
                            lhsT=xt[:, ds, ts * P : (ts + 1) * P],
                            rhs=wg_sb[:, ds, :],
                            start=(ds == 0),
                            stop=(ds == DS - 1),
                        )
                    logit = g_pool.tile([P, E], F32, name="logit")
                    nc.vector.tensor_add(logit[:], gps[:], bg_sb[:])
                    m1 = g_pool.tile([P, 1], F32, name="m1")
                    nc.vector.reduce_max(m1[:], logit[:], axis=AX)
                    nm1 = g_pool.tile([P, 1], F32, name="nm1")
                    nc.vector.tensor_scalar_mul(nm1[:], m1[:], -1.0)
                    pexp = g_pool.tile([P, E], F32, name="pexp")
                    nc.scalar.activation(pexp[:], logit[:], ACTF.Exp, bias=nm1[:])
                    ssum = g_pool.tile([P, 1], F32, name="ssum")
                    nc.vector.reduce_sum(ssum[:], pexp[:], axis=AX)
                    rs = g_pool.tile([P, 1], F32, name="rs")
                    nc.vector.reciprocal(rs[:], ssum[:])
                    # knock out the argmax, then top-2 = (logit >= 2nd max)
                    eqb = g_pool.tile([P, E], F32, name="eqb")
                    nc.vector.tensor_scalar(
                        eqb[:], logit[:], m1[:], 1e30, ALU.is_equal, ALU.mult
                    )
                    msk = g_pool.tile([P, E], F32, name="msk")
                    nc.vector.tensor_sub(msk[:], logit[:], eqb[:])
                    m2 = g_pool.tile([P, 1], F32, name="m2")
                    nc.vector.reduce_max(m2[:], msk[:], axis=AX)
                    ge = g_pool.tile([P, E], F32, name="ge")
                    nc.vector.tensor_scalar(ge[:], logit[:], m2[:], None, ALU.is_ge)
                    wsel = g_pool.tile([P, E], F32, name="wsel")
                    nc.vector.tensor_mul(wsel[:], pexp[:], ge[:])
                    nc.vector.tensor_mul(wsel[:], wsel[:], sel_sb[:])
                    wred = g_pool.tile([P, 1], F32, name="wred")
                    nc.vector.reduce_sum(wred[:], wsel[:], axis=AX)
                    nc.vector.tensor_tensor(
                        wcol[:, ts : ts + 1], wred[:], rs[:], ALU.mult
                    )

                # --- layer 1: hT[f, t] = gelu_tanh(W1.T x + b1), bf16 out
                h_sb = h_pool.tile([P, FS, TB], BF16)
                for fs in range(FS):
                    hps = psh_pool.tile([P, TB], F32)
                    for ds in range(DS):
                        nc.tensor.matmul(
                            hps[:],
                            lhsT=w1_sb[:, ds, fs * P : (fs + 1) * P],
                            rhs=xtbf[:, ds, :],
                            start=(ds == 0),
                            stop=(ds == DS - 1),
                        )
                    nc.scalar.activation(
                        h_sb[:, fs, :],
                        hps[:],
                        ACTF.Gelu_apprx_tanh,
                        bias=b1_sb[:, fs : fs + 1],
                    )

                # --- layer 2: out[t, d] = (hT.T @ W2 + b2) * w
                for ts in range(TSUB):
                    for dh in range(D // 512):
                        ops_ = pso_pool.tile([P, 512], F32)
                        for fs in range(FS):
                            nc.tensor.matmul(
                                ops_[:],
                                lhsT=h_sb[:, fs, ts * P : (ts + 1) * P],
                                rhs=w2_sb[:, fs, dh * 512 : (dh + 1) * 512],
                                start=(fs == 0),
                                stop=(fs == FS - 1),
                            )
                        osb = o_pool.tile([P, 512], F32)
                        nc.vector.tensor_add(
                            osb[:], ops_[:], b2_sb[:, dh * 512 : (dh + 1) * 512]
                        )
                        nc.vector.tensor_scalar_mul(osb[:], osb[:], wcol[:, ts : ts + 1])
                        nc.sync.dma_start(
                            out[t0 + ts * P : t0 + (ts + 1) * P, dh * 512 : (dh + 1) * 512],
                            osb[:],
                        )
    return nc


def kernel(x, Wg, bg, W1, b1, W2, b2):
    B, S, Dx = x.shape
    assert (B * S, Dx) == (T, D)
    xf = np.ascontiguousarray(x.reshape(T, D).astype(np.float32, copy=False))
    xT = np.ascontiguousarray(xf.T)

    if "nc" not in _CACHE:
        _CACHE["nc"] = _build_nc()
    nc = _CACHE["nc"]

    bf16 = ml_dtypes.bfloat16
    in_maps = []
    for e in range(NCORES):
        sel = np.zeros((P, E), dtype=np.float32)
        sel[:, e] = 1.0
        in_maps.append(
            {
                "xT": xT,
                "w1": np.ascontiguousarray(W1[e]).astype(bf16),
                "w2": np.ascontiguousarray(W2[e]).astype(bf16),
                "b1": np.ascontiguousarray(b1[e]).astype(np.float32),
                "b2r": np.broadcast_to(
                    b2[e].astype(np.float32), (P, D)
                ).copy(),
                "wg": np.ascontiguousarray(Wg).astype(np.float32),
                "bgr": np.broadcast_to(bg.astype(np.float32), (P, E)).copy(),
                "sel": sel,
            }
        )

    res = run_bass_kernel_spmd(nc, in_maps, core_ids=list(range(NCORES)))
    LAST["nc"] = nc
    LAST["in_maps"] = in_maps
    LAST["exec_time_ns"] = res.exec_time_ns

    acc = np.zeros((T, D), dtype=np.float32)
    for c in range(NCORES):
        acc += res.results[c]["out"]
    return acc.reshape(B, S, Dx)
